# revision 8
# baseline (speedup 1.0000x reference)
"""Trainium2 Bass kernel for nn_NearTransEncoder (ragged packed 4-layer encoder).

Self-contained: hardcodes shapes & sharding (data-parallel: 64 batches,
8 batches per NeuronCore). The ragged pack is expressed as one-hot gather
matmuls; all GEMMs run in fp16 with f32 PSUM accumulation.
"""
import sys
sys.path.insert(0, "/opt/trn_rl_repo")

import json
import types
import numpy as np
from contextlib import ExitStack

import concourse.bass as bass
import concourse.tile as tile
from concourse import mybir
from concourse.bass_utils import run_bass_kernel_spmd
from concourse.masks import make_identity

fp32 = mybir.dt.float32
fp16 = mybir.dt.float16
AF = mybir.ActivationFunctionType

NH, DK, D, DI, NLAY = 8, 64, 512, 2048, 4
VOCAB, REP, S, NNEAR = 512, 1, 12, 32
B, NC_ = 64, 8
BPC = B // NC_            # batches per core
LP = 256                  # padded per-batch length
NT = BPC * LP // 128      # 16 token tiles of 128 per core
EPS = 1e-6
INVT = 0.125
NEG = -1e9


# ---------------- BIR sync legalizer (walrus allows max 1 wait/instr) ----
def _legalize_bir_json(bj: bytes) -> bytes:
    m = json.loads(bj)
    proxy_id = None
    proxy_name = None
    for sid, names in m.get("ant_sem_names", {}).items():
        if "monotonic_0" in names:
            proxy_id = int(sid)
            proxy_name = "monotonic_0"
    proxy_count = 0
    uid = 0
    for fn in m["functions"]:
        for blk in fn["blocks"]:
            out = []
            for ins in blk["instructions"]:
                si = ins.get("sync_info")
                if not si:
                    out.append(ins)
                    continue
                waits = si.get("on_wait") or []
                upds = si.get("on_update") or []
                assert len(upds) <= 1, (
                    f"{ins.get('name')}: {len(upds)} sync updates"
                )
                if len(waits) <= 1:
                    out.append(ins)
                    continue
                debug = ins.get("debug", 0)
                if ins.get("opcode") == "DMACopy":
                    assert proxy_id is not None, "no monotonic_0 semaphore"
                    for w in waits:
                        uid += 1
                        out.append({
                            "debug": debug, "engine": "Pool", "ins": [],
                            "name": f"legw-{uid}", "opcode": "EventSemaphore",
                            "outs": [],
                            "sync_info": {"on_update": [], "on_wait": [w]},
                        })
                    proxy_count += 1
                    uid += 1
                    out.append({
                        "debug": debug, "engine": "Pool", "ins": [],
                        "name": f"legu-{uid}", "opcode": "EventSemaphore",
                        "outs": [],
                        "sync_info": {
                            "on_update": [{
                                "ant_name": proxy_name, "id": proxy_id,
                                "sync_type": "semaphore",
                                "update_mode": "sem-inc", "update_value": 1,
                            }],
                            "on_wait": [],
                        },
                    })
                    si["on_wait"] = [{
                        "ant_name": proxy_name, "id": proxy_id,
                        "sync_type": "semaphore",
                        "wait_mode": "sem-ge-imm", "wait_value": proxy_count,
                    }]
                    out.append(ins)
                else:
                    engine = ins["engine"]
                    for w in waits[:-1]:
                        uid += 1
                        out.append({
                            "debug": debug, "engine": engine, "ins": [],
                            "name": f"legw-{uid}", "opcode": "EventSemaphore",
                            "outs": [],
                            "sync_info": {"on_update": [], "on_wait": [w]},
                        })
                    si["on_wait"] = waits[-1:]
                    out.append(ins)
            blk["instructions"] = out
    return json.dumps(m).encode()


def _patch_nc(nc):
    orig = nc.to_json_bytes

    def patched(self, *a, **kw):
        return _legalize_bir_json(orig(*a, **kw))

    nc.to_json_bytes = types.MethodType(patched, nc)
    return nc


# ---------------- host-side prep ----------------------------------------
def _sinusoid(n, d):
    pos = np.arange(n)[:, None].astype(np.float64)
    i = np.arange(d)[None, :]
    angle = pos / np.power(10000.0, 2 * (i // 2) / d)
    tab = np.zeros((n, d))
    tab[:, 0::2] = np.sin(angle[:, 0::2])
    tab[:, 1::2] = np.cos(angle[:, 1::2])
    return tab.astype(np.float32)


def _host_prep(inputs):
    poi_type = np.asarray(inputs["poi_type"]).astype(np.int64)     # [NPOI,S]
    loc_emb = np.asarray(inputs["loc_emb"], np.float32)            # [NPOI,D]
    npl = np.asarray(inputs["near_pois_num_list"]).astype(np.int64)
    ttn = np.asarray(inputs["type_token_num"]).astype(np.int64)
    emb = np.asarray(inputs["emb"], np.float32)                    # [V,D]
    Wq = np.asarray(inputs["Wq"], np.float32)
    Wk = np.asarray(inputs["Wk"], np.float32)
    Wv = np.asarray(inputs["Wv"], np.float32)
    Wo = np.asarray(inputs["Wo"], np.float32)
    W1 = np.asarray(inputs["W1"], np.float32)
    W2 = np.asarray(inputs["W2"], np.float32)
    bo = np.asarray(inputs["bo"], np.float32)
    b1 = np.asarray(inputs["b1"], np.float32)
    b2 = np.asarray(inputs["b2"], np.float32)

    ends = np.cumsum(npl)
    starts = ends - npl
    seq_len = np.array([1 + int(ttn[starts[b]:ends[b]].sum())
                        for b in range(B)], np.int64)
    L = int(seq_len.max())
    assert L <= LP, f"L={L} exceeds padded length {LP}"
    poi_idx = np.zeros((B, LP), np.int64)
    tok_idx = np.zeros((B, LP), np.int64)
    for b in range(B):
        pos = 1
        for p in range(int(starts[b]), int(ends[b])):
            t = int(ttn[p])
            poi_idx[b, pos:pos + t] = p
            tok_idx[b, pos:pos + t] = np.arange(t)
            pos += t

    f16 = np.float16
    emb16 = emb.astype(f16).reshape(4, 128, D).transpose(1, 0, 2) \
               .reshape(128, 4 * D)
    pos16 = np.ascontiguousarray(_sinusoid(200, D)[:S].astype(f16))
    wq16 = Wq.astype(f16).reshape(NLAY, 4, 128, 4, 128) \
             .transpose(0, 2, 1, 3, 4).reshape(NLAY * 128, 2048)
    wk16 = Wk.astype(f16).reshape(NLAY, 4, 128, 4, 128) \
             .transpose(0, 2, 1, 3, 4).reshape(NLAY * 128, 2048)
    wv16 = Wv.astype(f16).reshape(NLAY, 4, 128, D) \
             .transpose(0, 2, 1, 3).reshape(NLAY * 128, 2048)
    wo16 = Wo.astype(f16).reshape(NLAY, 4, 128, D) \
             .transpose(0, 2, 1, 3).reshape(NLAY * 128, 2048)
    w116 = W1.astype(f16).reshape(NLAY, 4, 128, 16, 128) \
             .transpose(0, 2, 1, 3, 4).reshape(NLAY * 128, 8192)
    w216 = W2.astype(f16).reshape(NLAY, 16, 128, D) \
             .transpose(0, 2, 1, 3).reshape(NLAY * 128, 8192)

    bo_nz = bool(np.any(bo))
    b1_nz = bool(np.any(b1))
    b2_nz = bool(np.any(b2))
    bo16 = np.ascontiguousarray(bo.astype(f16))                 # [NLAY, D]
    b216 = np.ascontiguousarray(b2.astype(f16))                 # [NLAY, D]
    b1t = b1.reshape(NLAY, 16, 128).transpose(0, 2, 1) \
            .reshape(NLAY * 128, 16).astype(np.float32)
    b1t = np.ascontiguousarray(b1t)

    in_maps = []
    for ci in range(NC_):
        A = np.zeros((NT, 4, 128, 128), f16)    # [tile, vchunk, vin, tok]
        sp = np.zeros((NT, S, 128), f16)        # [tile, pos,   tok]
        sl = np.zeros((NT, NNEAR, 128), f16)    # [tile, local, tok]
        lc = np.zeros((NNEAR, BPC, D), f16)
        mk = np.full((128, 2 * BPC), NEG, np.float32)
        for bb in range(BPC):
            b = ci * BPC + bb
            A[bb * 2, REP // 128, REP % 128, 0] = 1.0       # [REP] slot 0
            sl_b = int(seq_len[b])
            ls = np.arange(1, sl_b)
            tiles = bb * 2 + ls // 128
            ps = ls % 128
            vs = poi_type[poi_idx[b, ls], tok_idx[b, ls]]
            A[tiles, vs // 128, vs % 128, ps] = 1.0
            sp[tiles, tok_idx[b, ls], ps] = 1.0
            sl[tiles, poi_idx[b, ls] - starts[b], ps] = 1.0
            n = int(npl[b])
            lc[:n, bb, :] = loc_emb[starts[b]:ends[b]].astype(f16)
            for kt in range(2):
                nvalid = min(max(sl_b - kt * 128, 0), 128)
                mk[:nvalid, bb * 2 + kt] = 0.0
        im = {
            "sel_emb": np.ascontiguousarray(
                A.transpose(0, 2, 1, 3).reshape(NT * 128, 512)),
            "sel_pos": np.ascontiguousarray(sp.reshape(NT * S, 128)),
            "sel_loc": np.ascontiguousarray(sl.reshape(NT * NNEAR, 128)),
            "embt": emb16, "post": pos16,
            "loct": np.ascontiguousarray(lc.reshape(NNEAR, BPC * D)),
            "maskb": mk,
            "wq": wq16, "wk": wk16, "wv": wv16, "wo": wo16,
            "w1": w116, "w2": w216,
        }
        if bo_nz:
            im["bod"] = bo16
        if b1_nz:
            im["b1d"] = b1t
        if b2_nz:
            im["b2d"] = b216
        in_maps.append(im)
    return {
        "in_maps": in_maps, "seq_len": seq_len, "L": L,
        "flags": (bo_nz, b1_nz, b2_nz),
    }


# ---------------- device program ----------------------------------------
def _build(bo_nz, b1_nz, b2_nz):
    nc = bass.Bass()
    sel_emb_d = nc.declare_dram_parameter("sel_emb", [NT * 128, 512], fp16,
                                          isOutput=False)
    sel_pos_d = nc.declare_dram_parameter("sel_pos", [NT * S, 128], fp16,
                                          isOutput=False)
    sel_loc_d = nc.declare_dram_parameter("sel_loc", [NT * NNEAR, 128], fp16,
                                          isOutput=False)
    emb_d = nc.declare_dram_parameter("embt", [128, 4 * D], fp16,
                                      isOutput=False)
    pos_d = nc.declare_dram_parameter("post", [S, D], fp16, isOutput=False)
    loc_d = nc.declare_dram_parameter("loct", [NNEAR, BPC * D], fp16,
                                      isOutput=False)
    mask_d = nc.declare_dram_parameter("maskb", [128, 2 * BPC], fp32,
                                       isOutput=False)
    wq_d = nc.declare_dram_parameter("wq", [NLAY * 128, 2048], fp16,
                                     isOutput=False)
    wk_d = nc.declare_dram_parameter("wk", [NLAY * 128, 2048], fp16,
                                     isOutput=False)
    wv_d = nc.declare_dram_parameter("wv", [NLAY * 128, 2048], fp16,
                                     isOutput=False)
    wo_d = nc.declare_dram_parameter("wo", [NLAY * 128, 2048], fp16,
                                     isOutput=False)
    w1_d = nc.declare_dram_parameter("w1", [NLAY * 128, 8192], fp16,
                                     isOutput=False)
    w2_d = nc.declare_dram_parameter("w2", [NLAY * 128, 8192], fp16,
                                     isOutput=False)
    if bo_nz:
        bo_d = nc.declare_dram_parameter("bod", [NLAY, D], fp16,
                                         isOutput=False)
    if b1_nz:
        b1_d = nc.declare_dram_parameter("b1d", [NLAY * 128, 16], fp32,
                                         isOutput=False)
    if b2_nz:
        b2_d = nc.declare_dram_parameter("b2d", [NLAY, D], fp16,
                                         isOutput=False)
    xout_d = nc.declare_dram_parameter("xout", [NT * 128, D], fp32,
                                       isOutput=True)

    with tile.TileContext(nc) as tc, ExitStack() as ctx:
        P = lambda name, bufs, space=None: ctx.enter_context(
            tc.tile_pool(name=name, bufs=bufs, **(
                {"space": space} if space else {})))
        sel_p = P("sel", 3)
        sp_p = P("spp", 3)
        sl_p = P("slp", 3)
        c_emb = P("cemb", 1)
        c_pos = P("cpos", 1)
        c_loc = P("cloc", 1)
        c_msk = P("cmsk", 1)
        c_id = P("cid", 1)
        c_cst = P("ccst", 1)
        x16_p = P("x16", 1)
        xt_p = P("xt", 1)
        ot_p = P("ot", 1)
        qk_p = P("qk", 2)
        v_p = P("vp", 2)
        e_p = P("ep", 9)
        o16_p = P("o16", 2)
        xr_p = P("xr", 3)
        ln_p = P("ln", 12)
        out_p = P("outp", 3)
        h_p = P("hp", 2)
        wq_p = P("wqp", 1)
        wk_p = P("wkp", 1)
        wv_p = P("wvp", 1)
        wo_p = P("wop", 1)
        w1_p = P("w1p", 1)
        w2_p = P("w2p", 1)
        if bo_nz or b2_nz:
            c_ones = P("cones", 1)
        if bo_nz:
            bo_p = P("bop", 1)
        if b1_nz:
            b1_p = P("b1p", 1)
        if b2_nz:
            b2_p = P("b2p", 1)
        ps_big = P("psb", 2, "PSUM")
        ps_qk = P("psq", 2, "PSUM")
        ps_sm = P("pss", 2, "PSUM")

        # constants
        ident = c_id.tile([128, 128], fp16)
        make_identity(nc, ident[:])
        cst_s = c_cst.tile([128, 2], fp32)      # [:,0]=eps, [:,1]=0.0
        nc.gpsimd.memset(cst_s[:, 0:1], EPS)
        nc.gpsimd.memset(cst_s[:, 1:2], 0.0)
        emb_s = c_emb.tile([128, 4 * D], fp16)
        nc.sync.dma_start(emb_s[:], emb_d[:])
        pos_s = c_pos.tile([S, D], fp16)
        nc.sync.dma_start(pos_s[:], pos_d[:])
        loc_s = c_loc.tile([NNEAR, BPC * D], fp16)
        nc.sync.dma_start(loc_s[:], loc_d[:])
        mask_s = c_msk.tile([128, 2 * BPC], fp32)
        nc.sync.dma_start(mask_s[:], mask_d[:])
        if bo_nz or b2_nz:
            ones_s = c_ones.tile([1, 128], fp16)
            nc.gpsimd.memset(ones_s[:], 1.0)

        X16 = x16_p.tile([128, NT * D], fp16)     # [tok%128, tile*512+d]
        XT = xt_p.tile([128, 4 * 2048], fp16)     # [d%128, kc*2048+tok]
        OT = ot_p.tile([128, 4 * 2048], fp16)

        def x16s(i):
            return X16[:, i * D:(i + 1) * D]

        def x16c(i, c):
            return X16[:, i * D + c * 128: i * D + (c + 1) * 128]

        def xts(kc, t0, n):
            return XT[:, kc * 2048 + t0: kc * 2048 + t0 + n]

        def ots(kc, t0, n):
            return OT[:, kc * 2048 + t0: kc * 2048 + t0 + n]

        def emit_ln(src_ap, dst_ap):
            st6 = ln_p.tile([128, 6], fp32)
            nc.vector.bn_stats(st6[:], src_ap)
            mv = ln_p.tile([128, 2], fp32)
            nc.vector.bn_aggr(mv[:], st6[:])
            std = ln_p.tile([128, 1], fp32)
            nc.scalar.activation(std[:], mv[:, 1:2], AF.Sqrt,
                                 bias=cst_s[:, 0:1])
            rstd = ln_p.tile([128, 1], fp32)
            nc.vector.reciprocal(rstd[:], std[:])
            mr = ln_p.tile([128, 1], fp32)
            nc.vector.tensor_scalar_mul(mr[:], mv[:, 0:1], rstd[:])
            nmr = ln_p.tile([128, 1], fp32)
            nc.vector.tensor_scalar_mul(nmr[:], mr[:], -1.0)
            nc.scalar.activation(dst_ap, src_ap, AF.Identity,
                                 bias=nmr[:], scale=rstd[:])

        def emit_tr(src_ap, dst_ap):
            pt = ps_sm.tile([128, 128], fp16, tag="sm")
            nc.tensor.transpose(pt[:], src_ap, ident[:])
            nc.vector.tensor_copy(dst_ap, pt[:])

        def emit_tr_tile(i):
            for c in range(4):
                emit_tr(x16c(i, c), xts(c, i * 128, 128))

        # ---------------- pack + initial LN + initial XT ----------------
        prev = None
        for i in range(NT):
            bb = i // 2
            sel_s = sel_p.tile([128, 512], fp16)
            nc.sync.dma_start(sel_s[:], sel_emb_d[i * 128:(i + 1) * 128, :])
            sp_s = sp_p.tile([S, 128], fp16)
            nc.sync.dma_start(sp_s[:], sel_pos_d[i * S:(i + 1) * S, :])
            sl_s = sl_p.tile([NNEAR, 128], fp16)
            nc.sync.dma_start(sl_s[:],
                              sel_loc_d[i * NNEAR:(i + 1) * NNEAR, :])
            p = ps_big.tile([128, D], fp32, tag="big")
            for c in range(4):
                nc.tensor.matmul(p[:], sel_s[:, c * 128:(c + 1) * 128],
                                 emb_s[:, c * D:(c + 1) * D],
                                 start=(c == 0), stop=False)
            nc.tensor.matmul(p[:], sp_s[:], pos_s[:],
                             start=False, stop=False)
            nc.tensor.matmul(p[:], sl_s[:],
                             loc_s[:, bb * D:(bb + 1) * D],
                             start=False, stop=True)
            emit_ln(p[:], x16s(i))
            if prev is not None:
                emit_tr_tile(prev)
            prev = i
        emit_tr_tile(prev)

        # ---------------- layers ----------------------------------------
        for l in range(NLAY):
            r0, r1 = l * 128, (l + 1) * 128
            wq_s = wq_p.tile([128, 2048], fp16)
            nc.sync.dma_start(wq_s[:], wq_d[r0:r1, :])
            wk_s = wk_p.tile([128, 2048], fp16)
            nc.sync.dma_start(wk_s[:], wk_d[r0:r1, :])
            wv_s = wv_p.tile([128, 2048], fp16)
            nc.sync.dma_start(wv_s[:], wv_d[r0:r1, :])
            wo_s = wo_p.tile([128, 2048], fp16)
            nc.sync.dma_start(wo_s[:], wo_d[r0:r1, :])
            w1_s = w1_p.tile([128, 8192], fp16)
            nc.sync.dma_start(w1_s[:], w1_d[r0:r1, :])
            w2_s = w2_p.tile([128, 8192], fp16)
            nc.sync.dma_start(w2_s[:], w2_d[r0:r1, :])
            if bo_nz:
                bo_s = bo_p.tile([1, D], fp16)
                nc.sync.dma_start(bo_s[:], bo_d[l:l + 1, :])
            if b1_nz:
                b1_s = b1_p.tile([128, 16], fp32)
                nc.sync.dma_start(b1_s[:], b1_d[r0:r1, :])
            if b2_nz:
                b2_s = b2_p.tile([1, D], fp16)
                nc.sync.dma_start(b2_s[:], b2_d[l:l + 1, :])

            # ---- attention, per batch ----
            for bb in range(BPC):
                t0 = bb * 256
                qt_s = qk_p.tile([128, 1024], fp16)   # [dq%128, c*256+t]
                kt_s = qk_p.tile([128, 1024], fp16)
                for c in range(4):
                    pq = ps_qk.tile([128, 256], fp32, tag="qk")
                    for kc in range(4):
                        nc.tensor.matmul(
                            pq[:],
                            wq_s[:, kc * 512 + c * 128: kc * 512 + (c + 1) * 128],
                            xts(kc, t0, 256),
                            start=(kc == 0), stop=(kc == 3))
                    nc.vector.tensor_copy(qt_s[:, c * 256:(c + 1) * 256],
                                          pq[:])
                    pk = ps_qk.tile([128, 256], fp32, tag="qk")
                    for kc in range(4):
                        nc.tensor.matmul(
                            pk[:],
                            wk_s[:, kc * 512 + c * 128: kc * 512 + (c + 1) * 128],
                            xts(kc, t0, 256),
                            start=(kc == 0), stop=(kc == 3))
                    nc.vector.tensor_copy(kt_s[:, c * 256:(c + 1) * 256],
                                          pk[:])
                vb = v_p.tile([128, 2 * 520], fp16)   # [k%128, kt*520+h*65+x]
                for kt in range(2):
                    pv_ = ps_big.tile([128, D], fp32, tag="big")
                    for kc in range(4):
                        nc.tensor.matmul(
                            pv_[:],
                            xts(kc, t0 + kt * 128, 128),
                            wv_s[:, kc * 512:(kc + 1) * 512],
                            start=(kc == 0), stop=(kc == 3))
                    vbk = vb[:, kt * 520:(kt + 1) * 520].rearrange(
                        "p (h x) -> p h x", h=NH)
                    nc.gpsimd.memset(vbk[:, :, 0:1], 1.0)
                    nc.vector.tensor_copy(
                        vbk[:, :, 1:65],
                        pv_[:].rearrange("p (h x) -> p h x", h=NH))
                ehs = []
                for h in range(NH):
                    po = (h % 2) * 64
                    co = (h // 2) * 256
                    eh = e_p.tile([128, 512], fp16)   # [k%128, kt*256+q]
                    for kt in range(2):
                        ps_ = ps_qk.tile([128, 256], fp32, tag="qk")
                        nc.tensor.matmul(
                            ps_[:],
                            kt_s[po:po + 64, co + kt * 128: co + (kt + 1) * 128],
                            qt_s[po:po + 64, co:co + 256],
                            start=True, stop=True)
                        mcol = bb * 2 + kt
                        nc.scalar.activation(
                            eh[:, kt * 256:(kt + 1) * 256], ps_[:], AF.Exp,
                            bias=mask_s[:, mcol:mcol + 1], scale=INVT)
                    ehs.append(eh)
                o16 = o16_p.tile([128, 1024], fp16)   # [q%128, qt*512+d]
                for h in range(NH):
                    eh = ehs[h]
                    for qt in range(2):
                        pp = ps_sm.tile([128, 65], fp32, tag="sm")
                        for kt in range(2):
                            nc.tensor.matmul(
                                pp[:],
                                eh[:, kt * 256 + qt * 128: kt * 256 + (qt + 1) * 128],
                                vb[:, kt * 520 + h * 65: kt * 520 + (h + 1) * 65],
                                start=(kt == 0), stop=(kt == 1))
                        rec = ln_p.tile([128, 1], fp32)
                        nc.vector.reciprocal(rec[:], pp[:, 0:1])
                        nc.vector.tensor_scalar_mul(
                            o16[:, qt * 512 + h * 64: qt * 512 + (h + 1) * 64],
                            pp[:, 1:65], rec[:])
                for qt in range(2):
                    for c in range(4):
                        emit_tr(o16[:, qt * 512 + c * 128: qt * 512 + (c + 1) * 128],
                                ots(c, t0 + qt * 128, 128))

            # ---- Wo + residual + LN1 + XT rebuild ----
            prev = None
            for i in range(NT):
                po = ps_big.tile([128, D], fp32, tag="big")
                for kc in range(4):
                    nc.tensor.matmul(po[:], ots(kc, i * 128, 128),
                                     wo_s[:, kc * 512:(kc + 1) * 512],
                                     start=(kc == 0),
                                     stop=(kc == 3 and not bo_nz))
                if bo_nz:
                    nc.tensor.matmul(po[:], ones_s[:], bo_s[:],
                                     start=False, stop=True)
                xr = xr_p.tile([128, D], fp32)
                nc.vector.tensor_add(xr[:], x16s(i), po[:])
                emit_ln(xr[:], x16s(i))
                if prev is not None:
                    emit_tr_tile(prev)
                prev = i
            emit_tr_tile(prev)

            # ---- FFN ----
            last = (l == NLAY - 1)

            def emit_ffn1(g):
                ht = h_p.tile([128, 8192], fp16)      # [di%128, dc*512+t]
                for dc in range(16):
                    pf = ps_big.tile([128, D], fp32, tag="big")
                    for kc in range(4):
                        nc.tensor.matmul(
                            pf[:],
                            w1_s[:, kc * 2048 + dc * 128: kc * 2048 + (dc + 1) * 128],
                            xts(kc, g * 512, 512),
                            start=(kc == 0), stop=(kc == 3))
                    if b1_nz:
                        nc.scalar.activation(
                            ht[:, dc * 512:(dc + 1) * 512], pf[:], AF.Relu,
                            bias=b1_s[:, dc:dc + 1])
                    else:
                        nc.scalar.activation(
                            ht[:, dc * 512:(dc + 1) * 512], pf[:], AF.Relu,
                            bias=cst_s[:, 1:2])
                return ht

            def emit_ffn2(g, ht):
                for it in range(4):
                    i = g * 4 + it
                    pf2 = ps_big.tile([128, D], fp32, tag="big")
                    for kc in range(16):
                        nc.tensor.matmul(
                            pf2[:],
                            ht[:, kc * 512 + it * 128: kc * 512 + (it + 1) * 128],
                            w2_s[:, kc * 512:(kc + 1) * 512],
                            start=(kc == 0),
                            stop=(kc == 15 and not b2_nz))
                    if b2_nz:
                        nc.tensor.matmul(pf2[:], ones_s[:], b2_s[:],
                                         start=False, stop=True)
                    xr = xr_p.tile([128, D], fp32)
                    nc.vector.tensor_add(xr[:], x16s(i), pf2[:])
                    if last:
                        xo = out_p.tile([128, D], fp32)
                        emit_ln(xr[:], xo[:])
                        nc.sync.dma_start(
                            xout_d[i * 128:(i + 1) * 128, :], xo[:])
                    else:
                        emit_ln(xr[:], x16s(i))

            pend = None
            for g in range(4):
                ht = emit_ffn1(g)
                if pend is not None:
                    emit_ffn2(pend[0], pend[1])
                pend = (g, ht)
            emit_ffn2(pend[0], pend[1])
            if not last:
                for i in range(NT):
                    emit_tr_tile(i)
    return _patch_nc(nc)


_NC_CACHE = {}


def kernel(**inputs):
    prep = _host_prep(inputs)
    flags = prep["flags"]
    if flags not in _NC_CACHE:
        _NC_CACHE[flags] = _build(*flags)
    nc = _NC_CACHE[flags]
    res = run_bass_kernel_spmd(nc, prep["in_maps"], list(range(NC_)),
                               trace=False)
    L = prep["L"]
    seq_len = prep["seq_len"]
    out = np.empty((B, L, D), np.float32)
    for ci in range(NC_):
        xo = np.asarray(res.results[ci]["xout"]).reshape(BPC, LP, D)
        out[ci * BPC:(ci + 1) * BPC] = xo[:, :L, :]
    attn_mask = np.arange(L)[None, :] < seq_len[:, None]
    return out, attn_mask
